# revision 72
# baseline (speedup 1.0000x reference)
"""Depth-aware 3x3 conv (Wang & Neumann depth-similarity modulated conv) on
8 Trainium2 NeuronCores, batch-parallel (1 image per core).

out[b,o,h,w] = sum_{c,k} weight[o,c,k] * fd[b,k,h,w] * xpatch[b,c,k,h,w] + bias
fd[k,p] = exp(-8.3 * |depth[p + delta_k] - depth[p]|)   (zero-padded patches)

Device-side formulation per core (image [64, 256, 256] fp16):
- Padded plane flattened: q = (h+1)*258 + (w+1), NP = 258*258.
- 9 taps delta(kh,kw) = (kh-1)*258 + (kw-1), paired so each pair's two taps
  live in partition halves of one [128, N] tile:
    pair x tiles: T1 = [x ; x@+1], T2 = [x ; x@+256]
    pairs: A=(t0,t1) on T1, B=(t7,t8) on T1, C=(t2,t3) on T2, D=(t5,t6) on T2
    center t4 unmodulated (fd=1) from T1 lower rows.
  T1 is one 3-dim DMA; T2: upper half DMA, lower half (plain x, duplicate
  of T1 lower) DVE-copied to save DMA-engine bandwidth.
- fd computed packed [72 = 9 taps x 8 segs, 512] fp16 (sub on DVE, abs+exp
  on ACT), scattered to a DRAM grid in ONE DMA. Replication to 64 channel
  rows per tap: pairs A/D via ONE pixel-aligned step-0 fanout DMA
  [128, 4096] each; pairs B/C via PE K=2 selector matmuls (fd rows -> PSUM,
  output-column cost only) + wide ACT psum->sbuf copies, trading DMA for
  PE/ACT headroom.
- modulate: DVE tensor_tensor IN PLACE over the fd tile (all-fp16 2x mode),
  two 2048-col pieces per pair, h=0 pieces for all pairs emitted first.
- PE: per 512-px group, 5 matmuls (4 pairs K=128 + center K=64) accumulate
  into PSUM [64, 1024] tiles (2 groups); ACT evicts 1024-wide with
  per-partition bias -> fp16.
- Software-pipelined emission (fd-gen chain 2 chunks ahead, x tiles/fd
  replication 1 chunk ahead of modulate/conv/evict) across split DMA issue
  queues (SP: x/fdu, ACT: depth loads+scatter+out, Pool SWDGE: fd fanout
  reads) so no in-order queue serializes production behind consumption.
TimelineSim: 284.6us/core (staged baseline: 593us).
"""
import numpy as np

import concourse.bacc as bacc
import concourse.bass as bass
import concourse.mybir as mybir
import concourse.tile as tile
from concourse.bass_utils import run_bass_kernel_spmd

F16 = mybir.dt.float16
F32 = mybir.dt.float32

B, C, H, W = 8, 64, 256, 256
Hp, Wp = H + 2, W + 2          # 258
NP = Hp * Wp                   # 66564
ALPHA = 8.3

CH = 4096                      # output pixels per chunk
NCHUNK = -(-NP // CH)          # 17 (out grid 69632, host slices)
WT = CH + 520                  # x2 tile width (halo for tap shifts)
SEG, SEGW = 8, CH // 8         # fd packing: [72, 512]

# line slacks (elements)
XSL, XSH = 512, 4608           # x line: reads [q0-260, q0+CH+516)
DSL, DSH = 512, 4608           # depth line
XW = XSL + NP + XSH
DW = DSL + NP + DSH
OUTW = NCHUNK * CH             # 69632

REGW = CH + 16                 # fd grid region width (per parity)
FDW = 6 * REGW

# tap id t = kh*3+kw, delta = (kh-1)*258 + (kw-1)
DELTA = [(kh - 1) * Wp + (kw - 1) for kh in range(3) for kw in range(3)]
# pairs (ta, tb): tb's shift baked into the x tile's upper half
PAIRS = [(0, 1, 0), (7, 8, 0), (2, 3, 1), (5, 6, 1)]  # (ta, tb, tile_idx)
UPPER_SHIFT = [1, 256]  # T1, T2
# fd replication route per pair: 'dma' (step-0 fanout read) or PE selector
# matmul + psum->sbuf copy on 'act' / 'pool'
FR_ROUTE = ["dma", "act", "act", "dma"]
NREG = 6                       # fd grid parity regions (pipeline depth)


def _build_nc():
    nc = bacc.Bacc("TRN2", target_bir_lowering=False, debug=False, num_devices=8)
    x_line = nc.declare_dram_parameter("x_line", [C, XW], F16, isOutput=False)
    d_line = nc.declare_dram_parameter("d_line", [1, DW], F16, isOutput=False)
    wts = nc.declare_dram_parameter("wts", [128, 336], F16, isOutput=False)
    bias = nc.declare_dram_parameter("bias", [64, 1], F32, isOutput=False)
    out_l = nc.declare_dram_parameter("out_line", [C, OUTW], F16, isOutput=True)

    x_t = x_line.ap().tensor
    d_t = d_line.ap().tensor
    fd_dram = nc.dram_tensor("fd_scratch", [9, FDW], F16)
    fd_t = fd_dram.ap().tensor

    with tile.TileContext(nc) as tc:
        with (
            tc.tile_pool(name="const", bufs=1) as cpool,
            tc.tile_pool(name="xt", bufs=6) as xpool,
            tc.tile_pool(name="fdgen", bufs=3) as gpool,
            tc.tile_pool(name="fduP", bufs=2) as upool,
            tc.tile_pool(name="frep", bufs=12) as fpool,
            tc.tile_pool(name="ost", bufs=2) as opool,
            tc.tile_pool(name="ps", bufs=2, space="PSUM") as pspool,
            tc.tile_pool(name="ps2", bufs=2, space="PSUM") as ps2pool,
        ):
            wt_sb = cpool.tile([128, 5 * 64], F16, tag="w")
            nc.sync.dma_start(wt_sb[:], wts[:, 0:320])
            bias_sb = cpool.tile([64, 1], F32, tag="b")
            nc.sync.dma_start(bias_sb[:], bias[:])
            # selector lhsT for fd replication (hybrid routes): loaded from
            # the tail columns of the wts param (cols 320:322 hold selT)
            sel_sb = None
            if any(r != "dma" for r in FR_ROUTE):
                sel_sb = cpool.tile([66, 128], F16, tag="sel")
                for sbase in (0, 64):
                    nc.sync.dma_start(
                        sel_sb[sbase:sbase + 2, :],
                        bass.AP(wts.ap().tensor, 320, [[1, 2], [336, 128]]))

            # -- software-pipelined 3 stages:
            #   A-early(i): fd generation chain (depth loads, sub/abs/exp,
            #     scatter) -- 2 chunks ahead, small tiles.
            #   A-late(i): x tiles, fd replication (DMA fanout reads,
            #     selector matmuls + ACT copies, Pool broadcasts) -- 1 ahead.
            #   B(i): modulate, conv matmuls, evict, out.
            # Emission order keeps every in-order engine queue's production
            # work ahead of consumption work.
            g_state = {}
            state = {}

            def stage_ae(i):
                q0 = i * CH
                # fd generation (packed [72, 512])
                dp = gpool.tile([72, SEGW], F16, tag="dp")
                for kh in range(3):
                    nc.scalar.dma_start(
                        dp[kh * 24:(kh + 1) * 24, :],
                        bass.AP(d_t, DSL + q0 - 259 + kh * Wp,
                                [[1, 3], [SEGW, SEG], [1, SEGW]]))
                dc = gpool.tile([72, SEGW], F16, tag="dc")
                nc.scalar.dma_start(
                    dc[:],
                    bass.AP(d_t, DSL + q0,
                            [[0, 9], [SEGW, SEG], [1, SEGW]]))
                df = gpool.tile([72, SEGW], F16, tag="df")
                nc.vector.tensor_tensor(df[:], dp[:], dc[:],
                                        mybir.AluOpType.subtract)
                da = gpool.tile([72, SEGW], F16, tag="da")
                nc.scalar.activation(da[:], df[:],
                                     mybir.ActivationFunctionType.Abs)
                fdp = gpool.tile([72, SEGW], F16, tag="fdp")
                nc.scalar.activation(fdp[:], da[:],
                                     mybir.ActivationFunctionType.Exp,
                                     scale=-ALPHA)
                # scatter to DRAM fd grid: ONE DMA (per-parity region)
                reg = (i % NREG) * REGW
                nc.scalar.dma_start(
                    bass.AP(fd_t, reg,
                            [[FDW, 9], [SEGW, SEG], [1, SEGW]]),
                    fdp[:])
                g_state[i] = reg

            def stage_al(i):
                q0 = i * CH
                chw = min(CH, (-(-(NP - q0) // 512)) * 512)
                reg = g_state.pop(i)
                xbase = XSL + q0 - 260
                # x2 tiles: T1 by one 3-dim DMA; T2 upper by DMA, T2
                # lower (plain x, same data as T1 lower) by DVE copy
                xts = []
                xw = chw + 520
                for ti in range(2):
                    xt = xpool.tile([128, WT], F16, tag="x")
                    if ti == 0:
                        nc.sync.dma_start(
                            xt[:, 0:xw],
                            bass.AP(x_t, xbase,
                                    [[UPPER_SHIFT[0], 2], [XW, 64],
                                     [1, xw]]))
                    else:
                        nc.sync.dma_start(
                            xt[64:128, 0:xw],
                            bass.AP(x_t, xbase + UPPER_SHIFT[1],
                                    [[XW, 64], [1, xw]]))
                        nc.vector.tensor_copy(xt[0:64, 0:xw],
                                              xts[0][0:64, 0:xw])
                    xts.append(xt)

                # engine-routed pairs share one [66, CH] fd-row tile
                fdu = None
                ubase = {}
                eng_pairs = [g for g in range(4) if FR_ROUTE[g] != "dma"]
                if eng_pairs:
                    fdu = upool.tile([66, CH], F16, tag="fdu")
                    for slot, g in enumerate(eng_pairs[:2]):
                        ta, tb, _ = PAIRS[g]
                        base = 64 * slot
                        ubase[g] = base
                        nc.sync.dma_start(
                            fdu[base:base + 2, 0:chw],
                            bass.AP(fd_t, ta * FDW + reg,
                                    [[(tb - ta) * FDW, 2], [1, chw]]))
                # fd replication into fr tiles (all routes)
                frs = {}
                for g, (ta, tb, ti) in enumerate(PAIRS):
                    fr = fpool.tile([128, CH], F16, tag="fr")
                    frs[g] = fr
                    route = FR_ROUTE[g]
                    if route == "dma":
                        nc.gpsimd.dma_start(
                            fr[:, 0:chw],
                            bass.AP(fd_t, ta * FDW + reg,
                                    [[(tb - ta) * FDW, 2], [0, 64],
                                     [1, chw]]))
                    elif route == "half":
                        hw2 = chw // 2
                        nc.gpsimd.dma_start(
                            fr[:, 0:hw2],
                            bass.AP(fd_t, ta * FDW + reg,
                                    [[(tb - ta) * FDW, 2], [0, 64],
                                     [1, hw2]]))
                        base = ubase[g]
                        for j in range(hw2 // 512, chw // 512):
                            fps = ps2pool.tile([128, 512], F32, tag="fps")
                            nc.tensor.matmul(
                                fps[:], sel_sb[base:base + 2, :],
                                fdu[base:base + 2, j * 512:(j + 1) * 512],
                                start=True, stop=True)
                            nc.scalar.activation(
                                fr[:, j * 512:(j + 1) * 512], fps[:],
                                mybir.ActivationFunctionType.Identity)
                    elif route == "bcast":
                        base = ubase[g]
                        nc.gpsimd.partition_broadcast(
                            fr[0:64, 0:chw], fdu[base:base + 1, 0:chw])
                        nc.gpsimd.partition_broadcast(
                            fr[64:128, 0:chw], fdu[base + 1:base + 2, 0:chw])
                    else:  # selector matmuls + psum->sbuf copies on
                        # ACT ('act') or ACT+DVE alternating ('mix')
                        base = ubase[g]
                        for jj in range(chw // 1024):
                            fps = ps2pool.tile([128, 1024], F32,
                                               tag="fps")
                            for sub in range(2):
                                j = jj * 2 + sub
                                nc.tensor.matmul(
                                    fps[:, sub * 512:(sub + 1) * 512],
                                    sel_sb[base:base + 2, :],
                                    fdu[base:base + 2,
                                        j * 512:(j + 1) * 512],
                                    start=True, stop=True)
                            if route == "act" or jj % 2 == 0:
                                nc.scalar.activation(
                                    fr[:, jj * 1024:(jj + 1) * 1024], fps[:],
                                    mybir.ActivationFunctionType.Identity)
                            else:
                                nc.vector.tensor_copy(
                                    fr[:, jj * 1024:(jj + 1) * 1024], fps[:])
                state[i] = (chw, xts, frs)

            def stage_b(i):
                q0 = i * CH
                chw, xts, frs = state.pop(i)
                # modulate IN PLACE over fr; emit piece h=0 for all pairs
                # first so early conv j-groups unblock sooner
                mts = [frs[g] for g in range(4)]
                half = chw // 2
                for h in range(2):
                    for g, (ta, tb, ti) in enumerate(PAIRS):
                        m0 = 260 + DELTA[ta]
                        lo, hi = h * half, h * half + half
                        nc.vector.tensor_tensor(
                            frs[g][:, lo:hi], xts[ti][:, m0 + lo:m0 + hi],
                            frs[g][:, lo:hi], mybir.AluOpType.mult)

                # matmuls + eviction (evict 1024-wide: 2 j-groups/psum tile)
                ost = opool.tile([64, CH], F16, tag="o")
                for jj in range(chw // 1024):
                    ps = pspool.tile([64, 1024], F32)
                    for sub in range(2):
                        j = jj * 2 + sub
                        pslice = ps[:, sub * 512:(sub + 1) * 512]
                        for g in range(4):
                            nc.tensor.matmul(
                                pslice, wt_sb[:, g * 64:(g + 1) * 64],
                                mts[g][:, j * 512:(j + 1) * 512],
                                start=(g == 0), stop=False)
                        nc.tensor.matmul(
                            pslice, wt_sb[0:64, 256:320],
                            xts[0][0:64, 260 + j * 512: 260 + (j + 1) * 512],
                            start=False, stop=True)
                    nc.scalar.activation(
                        ost[:, jj * 1024:(jj + 1) * 1024], ps[:],
                        mybir.ActivationFunctionType.Identity,
                        bias=bias_sb[:], scale=1.0)
                nc.sync.dma_start(out_l[:, q0:q0 + chw], ost[:, 0:chw])

            for k in range(NCHUNK + 2):
                if k < NCHUNK:
                    stage_ae(k)
                if 1 <= k <= NCHUNK:
                    stage_al(k - 1)
                if k >= 2:
                    stage_b(k - 2)
    nc.compile()
    return nc


_NC_CACHE = None


def _get_nc():
    global _NC_CACHE
    if _NC_CACHE is None:
        _NC_CACHE = _build_nc()
    return _NC_CACHE


def kernel(x, depth, weight, bias):
    x = np.asarray(x, dtype=np.float32)
    depth = np.asarray(depth, dtype=np.float32)
    weight = np.asarray(weight, dtype=np.float32)
    bias_np = np.asarray(bias, dtype=np.float32)

    # host prep
    xl = np.zeros((B, C, XW), np.float16)
    xpad = np.zeros((B, C, Hp, Wp), np.float32)
    xpad[:, :, 1:257, 1:257] = x
    xl[:, :, XSL:XSL + NP] = xpad.reshape(B, C, NP).astype(np.float16)

    dl = np.zeros((B, 1, DW), np.float16)
    dpad = np.zeros((B, Hp, Wp), np.float32)
    dpad[:, 1:257, 1:257] = depth[:, 0]
    dl[:, 0, DSL:DSL + NP] = dpad.reshape(B, NP).astype(np.float16)

    wts = np.zeros((128, 336), np.float16)
    wts[0:64, 320] = 1.0    # selT col 0: partitions 0..63
    wts[64:128, 321] = 1.0  # selT col 1: partitions 64..127
    for g, (ta, tb, _) in enumerate(PAIRS):
        # lhsT[c, o] = weight[o, c, kh, kw]
        wts[0:64, g * 64:(g + 1) * 64] = \
            weight[:, :, ta // 3, ta % 3].T.astype(np.float16)
        wts[64:128, g * 64:(g + 1) * 64] = \
            weight[:, :, tb // 3, tb % 3].T.astype(np.float16)
    wts[0:64, 256:320] = weight[:, :, 1, 1].T.astype(np.float16)

    bias_col = bias_np.reshape(64, 1)

    nc = _get_nc()
    in_maps = [
        {"x_line": xl[b], "d_line": dl[b], "wts": wts, "bias": bias_col}
        for b in range(B)
    ]
    res = run_bass_kernel_spmd(nc, in_maps, list(range(B)))

    out = np.empty((B, C, H, W), np.float32)
    for b in range(B):
        ol = res.results[b]["out_line"][:, :NP].astype(np.float32)
        out[b] = ol.reshape(C, Hp, Wp)[:, 1:257, 1:257]
    return out


# revision 77
# speedup vs baseline: 3027.6770x; 3027.6770x over previous
"""Depth-aware 3x3 conv (Wang & Neumann depth-similarity modulated conv) on
8 Trainium2 NeuronCores, batch-parallel (1 image per core).

out[b,o,h,w] = sum_{c,k} weight[o,c,k] * fd[b,k,h,w] * xpatch[b,c,k,h,w] + bias
fd[k,p] = exp(-8.3 * |depth[p + delta_k] - depth[p]|)   (zero-padded patches)

Device-side formulation per core (image [64, 256, 256] fp16):
- Padded plane flattened: q = (h+1)*258 + (w+1), NP = 258*258.
- 9 taps delta(kh,kw) = (kh-1)*258 + (kw-1), paired so each pair's two taps
  live in partition halves of one [128, N] tile:
    pair x tiles: T1 = [x ; x@+1], T2 = [x ; x@+256]
    pairs: A=(t0,t1) on T1, B=(t7,t8) on T1, C=(t2,t3) on T2, D=(t5,t6) on T2
    center t4 unmodulated (fd=1) from T1 lower rows.
  T1 is one 3-dim DMA; T2: upper half DMA, lower half (plain x, duplicate
  of T1 lower) DVE-copied to save DMA-engine bandwidth.
- fd computed packed [72 = 9 taps x 8 segs, 512] fp16 (sub on DVE, abs+exp
  on ACT), scattered to a DRAM grid in ONE DMA. Replication to 64 channel
  rows per tap: pairs A/D via ONE pixel-aligned step-0 fanout DMA
  [128, 4096] each; pairs B/C via PE K=2 selector matmuls (fd rows -> PSUM,
  output-column cost only) + wide ACT psum->sbuf copies, trading DMA for
  PE/ACT headroom.
- modulate: DVE tensor_tensor IN PLACE over the fd tile (all-fp16 2x mode),
  two 2048-col pieces per pair, h=0 pieces for all pairs emitted first.
- PE: per 512-px group, 5 matmuls (4 pairs K=128 + center K=64) accumulate
  into PSUM [64, 1024] tiles (2 groups); ACT evicts 1024-wide with
  per-partition bias -> fp16.
- Software-pipelined emission (fd-gen chain 2 chunks ahead, x tiles/fd
  replication 1 chunk ahead of modulate/conv/evict) across split DMA issue
  queues (SP: x/fdu, ACT: depth loads+scatter+out, Pool SWDGE: fd fanout
  reads) so no in-order queue serializes production behind consumption.
TimelineSim: 284.6us/core (staged baseline: 593us).
"""
import numpy as np

import concourse.bacc as bacc
import concourse.bass as bass
import concourse.mybir as mybir
import concourse.tile as tile
from concourse.bass_utils import run_bass_kernel_spmd

F16 = mybir.dt.float16
F32 = mybir.dt.float32

B, C, H, W = 8, 64, 256, 256
Hp, Wp = H + 2, W + 2          # 258
NP = Hp * Wp                   # 66564
ALPHA = 8.3

CH = 4096                      # output pixels per chunk
NCHUNK = -(-NP // CH)          # 17 (out grid 69632, host slices)
WT = CH + 520                  # x2 tile width (halo for tap shifts)
SEG, SEGW = 8, CH // 8         # fd packing: [72, 512]

# line slacks (elements)
XSL, XSH = 512, 4608           # x line: reads [q0-260, q0+CH+516)
DSL, DSH = 512, 4608           # depth line
XW = XSL + NP + XSH
DW = DSL + NP + DSH
OUTW = NCHUNK * CH             # 69632

REGW = CH + 16                 # fd grid region width (per parity)
FDW = 6 * REGW

# tap id t = kh*3+kw, delta = (kh-1)*258 + (kw-1)
DELTA = [(kh - 1) * Wp + (kw - 1) for kh in range(3) for kw in range(3)]
# pairs (ta, tb): tb's shift baked into the x tile's upper half
PAIRS = [(0, 1, 0), (7, 8, 0), (2, 3, 1), (5, 6, 1)]  # (ta, tb, tile_idx)
UPPER_SHIFT = [1, 256]  # T1, T2
# fd replication route per pair: 'dma' (step-0 fanout read) or PE selector
# matmul + psum->sbuf copy on 'act' / 'pool'
FR_ROUTE = ["dma", "act", "act", "dma"]
NREG = 6                       # fd grid parity regions (pipeline depth)


def _build_nc():
    nc = bacc.Bacc("TRN2", target_bir_lowering=False, debug=False, num_devices=8)
    x_line = nc.declare_dram_parameter("x_line", [C, XW], F16, isOutput=False)
    d_line = nc.declare_dram_parameter("d_line", [1, DW], F16, isOutput=False)
    wts = nc.declare_dram_parameter("wts", [128, 336], F16, isOutput=False)
    bias = nc.declare_dram_parameter("bias", [64, 1], F32, isOutput=False)
    out_l = nc.declare_dram_parameter("out_line", [C, OUTW], F16, isOutput=True)

    x_t = x_line.ap().tensor
    d_t = d_line.ap().tensor
    fd_dram = nc.dram_tensor("fd_scratch", [9, FDW], F16)
    fd_t = fd_dram.ap().tensor

    with tile.TileContext(nc) as tc:
        with (
            tc.tile_pool(name="const", bufs=1) as cpool,
            tc.tile_pool(name="xt", bufs=6) as xpool,
            tc.tile_pool(name="fdgen", bufs=3) as gpool,
            tc.tile_pool(name="fduP", bufs=2) as upool,
            tc.tile_pool(name="frep", bufs=12) as fpool,
            tc.tile_pool(name="ost", bufs=2) as opool,
            tc.tile_pool(name="ps", bufs=2, space="PSUM") as pspool,
            tc.tile_pool(name="ps2", bufs=2, space="PSUM") as ps2pool,
        ):
            wt_sb = cpool.tile([128, 5 * 64], F16, tag="w")
            nc.sync.dma_start(wt_sb[:], wts[:, 0:320])
            bias_sb = cpool.tile([64, 1], F32, tag="b")
            nc.sync.dma_start(bias_sb[:], bias[:])
            # selector lhsT for fd replication (hybrid routes): loaded from
            # the tail columns of the wts param (cols 320:322 hold selT)
            sel_sb = None
            if any(r != "dma" for r in FR_ROUTE):
                sel_sb = cpool.tile([66, 128], F16, tag="sel")
                for sbase in (0, 64):
                    nc.sync.dma_start(
                        sel_sb[sbase:sbase + 2, :],
                        bass.AP(wts.ap().tensor, 320, [[1, 2], [336, 128]]))

            # -- software-pipelined 3 stages:
            #   A-early(i): fd generation chain (depth loads, sub/abs/exp,
            #     scatter) -- 2 chunks ahead, small tiles.
            #   A-late(i): x tiles, fd replication (DMA fanout reads,
            #     selector matmuls + ACT copies, Pool broadcasts) -- 1 ahead.
            #   B(i): modulate, conv matmuls, evict, out.
            # Emission order keeps every in-order engine queue's production
            # work ahead of consumption work.
            g_state = {}
            state = {}

            def stage_ae(i):
                q0 = i * CH
                # fd generation (packed [72, 512])
                dp = gpool.tile([72, SEGW], F16, tag="dp")
                for kh in range(3):
                    nc.scalar.dma_start(
                        dp[kh * 24:(kh + 1) * 24, :],
                        bass.AP(d_t, DSL + q0 - 259 + kh * Wp,
                                [[1, 3], [SEGW, SEG], [1, SEGW]]))
                dc = gpool.tile([72, SEGW], F16, tag="dc")
                nc.scalar.dma_start(
                    dc[:],
                    bass.AP(d_t, DSL + q0,
                            [[0, 9], [SEGW, SEG], [1, SEGW]]))
                df = gpool.tile([72, SEGW], F16, tag="df")
                nc.vector.tensor_tensor(df[:], dp[:], dc[:],
                                        mybir.AluOpType.subtract)
                da = gpool.tile([72, SEGW], F16, tag="da")
                nc.scalar.activation(da[:], df[:],
                                     mybir.ActivationFunctionType.Abs)
                fdp = gpool.tile([72, SEGW], F16, tag="fdp")
                nc.scalar.activation(fdp[:], da[:],
                                     mybir.ActivationFunctionType.Exp,
                                     scale=-ALPHA)
                # scatter to DRAM fd grid: ONE DMA (per-parity region)
                reg = (i % NREG) * REGW
                nc.scalar.dma_start(
                    bass.AP(fd_t, reg,
                            [[FDW, 9], [SEGW, SEG], [1, SEGW]]),
                    fdp[:])
                g_state[i] = reg

            def stage_al(i):
                q0 = i * CH
                chw = min(CH, (-(-(NP - q0) // 512)) * 512)
                reg = g_state.pop(i)
                xbase = XSL + q0 - 260
                # x2 tiles: T1 by one 3-dim DMA; T2 upper by DMA, T2
                # lower (plain x, same data as T1 lower) by DVE copy
                xts = []
                xw = chw + 520
                for ti in range(2):
                    xt = xpool.tile([128, WT], F16, tag="x")
                    if ti == 0:
                        nc.sync.dma_start(
                            xt[:, 0:xw],
                            bass.AP(x_t, xbase,
                                    [[UPPER_SHIFT[0], 2], [XW, 64],
                                     [1, xw]]))
                    else:
                        nc.sync.dma_start(
                            xt[64:128, 0:xw],
                            bass.AP(x_t, xbase + UPPER_SHIFT[1],
                                    [[XW, 64], [1, xw]]))
                        nc.vector.tensor_copy(xt[0:64, 0:xw],
                                              xts[0][0:64, 0:xw])
                    xts.append(xt)

                # engine-routed pairs share one [66, CH] fd-row tile
                fdu = None
                ubase = {}
                eng_pairs = [g for g in range(4) if FR_ROUTE[g] != "dma"]
                if eng_pairs:
                    fdu = upool.tile([66, CH], F16, tag="fdu")
                    for slot, g in enumerate(eng_pairs[:2]):
                        ta, tb, _ = PAIRS[g]
                        base = 64 * slot
                        ubase[g] = base
                        nc.sync.dma_start(
                            fdu[base:base + 2, 0:chw],
                            bass.AP(fd_t, ta * FDW + reg,
                                    [[(tb - ta) * FDW, 2], [1, chw]]))
                # fd replication into fr tiles (all routes)
                frs = {}
                for g, (ta, tb, ti) in enumerate(PAIRS):
                    fr = fpool.tile([128, CH], F16, tag="fr")
                    frs[g] = fr
                    route = FR_ROUTE[g]
                    if route == "dma":
                        nc.gpsimd.dma_start(
                            fr[:, 0:chw],
                            bass.AP(fd_t, ta * FDW + reg,
                                    [[(tb - ta) * FDW, 2], [0, 64],
                                     [1, chw]]))
                    elif route == "half":
                        hw2 = chw // 2
                        nc.gpsimd.dma_start(
                            fr[:, 0:hw2],
                            bass.AP(fd_t, ta * FDW + reg,
                                    [[(tb - ta) * FDW, 2], [0, 64],
                                     [1, hw2]]))
                        base = ubase[g]
                        for j in range(hw2 // 512, chw // 512):
                            fps = ps2pool.tile([128, 512], F32, tag="fps")
                            nc.tensor.matmul(
                                fps[:], sel_sb[base:base + 2, :],
                                fdu[base:base + 2, j * 512:(j + 1) * 512],
                                start=True, stop=True)
                            nc.scalar.activation(
                                fr[:, j * 512:(j + 1) * 512], fps[:],
                                mybir.ActivationFunctionType.Identity)
                    elif route == "bcast":
                        base = ubase[g]
                        nc.gpsimd.partition_broadcast(
                            fr[0:64, 0:chw], fdu[base:base + 1, 0:chw])
                        nc.gpsimd.partition_broadcast(
                            fr[64:128, 0:chw], fdu[base + 1:base + 2, 0:chw])
                    else:  # selector matmuls + psum->sbuf copies on
                        # ACT ('act') or ACT+DVE alternating ('mix')
                        base = ubase[g]
                        for jj in range(chw // 1024):
                            fps = ps2pool.tile([128, 1024], F32,
                                               tag="fps")
                            for sub in range(2):
                                j = jj * 2 + sub
                                nc.tensor.matmul(
                                    fps[:, sub * 512:(sub + 1) * 512],
                                    sel_sb[base:base + 2, :],
                                    fdu[base:base + 2,
                                        j * 512:(j + 1) * 512],
                                    start=True, stop=True)
                            if route == "act" or jj % 2 == 0:
                                nc.scalar.activation(
                                    fr[:, jj * 1024:(jj + 1) * 1024], fps[:],
                                    mybir.ActivationFunctionType.Identity)
                            else:
                                nc.vector.tensor_copy(
                                    fr[:, jj * 1024:(jj + 1) * 1024], fps[:])
                state[i] = (chw, xts, frs)

            def stage_b(i):
                q0 = i * CH
                chw, xts, frs = state.pop(i)
                # modulate IN PLACE over fr; emit piece h=0 for all pairs
                # first so early conv j-groups unblock sooner
                mts = [frs[g] for g in range(4)]
                half = chw // 2
                for h in range(2):
                    for g, (ta, tb, ti) in enumerate(PAIRS):
                        m0 = 260 + DELTA[ta]
                        lo, hi = h * half, h * half + half
                        nc.vector.tensor_tensor(
                            frs[g][:, lo:hi], xts[ti][:, m0 + lo:m0 + hi],
                            frs[g][:, lo:hi], mybir.AluOpType.mult)

                # matmuls + eviction (evict 1024-wide: 2 j-groups/psum tile)
                ost = opool.tile([64, CH], F16, tag="o")
                for jj in range(chw // 1024):
                    ps = pspool.tile([64, 1024], F32)
                    for sub in range(2):
                        j = jj * 2 + sub
                        pslice = ps[:, sub * 512:(sub + 1) * 512]
                        for g in range(4):
                            nc.tensor.matmul(
                                pslice, wt_sb[:, g * 64:(g + 1) * 64],
                                mts[g][:, j * 512:(j + 1) * 512],
                                start=(g == 0), stop=False)
                        nc.tensor.matmul(
                            pslice, wt_sb[0:64, 256:320],
                            xts[0][0:64, 260 + j * 512: 260 + (j + 1) * 512],
                            start=False, stop=True)
                    nc.scalar.activation(
                        ost[:, jj * 1024:(jj + 1) * 1024], ps[:],
                        mybir.ActivationFunctionType.Identity,
                        bias=bias_sb[:], scale=1.0)
                nc.sync.dma_start(out_l[:, q0:q0 + chw], ost[:, 0:chw])

            for k in range(NCHUNK + 2):
                if k < NCHUNK:
                    stage_ae(k)
                if 1 <= k <= NCHUNK:
                    stage_al(k - 1)
                if k >= 2:
                    stage_b(k - 2)
    nc.compile()
    return nc


_NC_CACHE = None


def _get_nc():
    global _NC_CACHE
    if _NC_CACHE is None:
        _NC_CACHE = _build_nc()
    return _NC_CACHE


def kernel(x, depth, weight, bias):
    x = np.asarray(x, dtype=np.float32)
    depth = np.asarray(depth, dtype=np.float32)
    weight = np.asarray(weight, dtype=np.float32)
    bias_np = np.asarray(bias, dtype=np.float32)

    # host prep
    xl = np.zeros((B, C, XW), np.float16)
    xpad = np.zeros((B, C, Hp, Wp), np.float32)
    xpad[:, :, 1:257, 1:257] = x
    xl[:, :, XSL:XSL + NP] = xpad.reshape(B, C, NP).astype(np.float16)

    dl = np.zeros((B, 1, DW), np.float16)
    dpad = np.zeros((B, Hp, Wp), np.float32)
    dpad[:, 1:257, 1:257] = depth[:, 0]
    dl[:, 0, DSL:DSL + NP] = dpad.reshape(B, NP).astype(np.float16)

    wts = np.zeros((128, 336), np.float16)
    wts[0:64, 320] = 1.0    # selT col 0: partitions 0..63
    wts[64:128, 321] = 1.0  # selT col 1: partitions 64..127
    for g, (ta, tb, _) in enumerate(PAIRS):
        # lhsT[c, o] = weight[o, c, kh, kw]
        wts[0:64, g * 64:(g + 1) * 64] = \
            weight[:, :, ta // 3, ta % 3].T.astype(np.float16)
        wts[64:128, g * 64:(g + 1) * 64] = \
            weight[:, :, tb // 3, tb % 3].T.astype(np.float16)
    wts[0:64, 256:320] = weight[:, :, 1, 1].T.astype(np.float16)

    bias_col = bias_np.reshape(64, 1)

    nc = _get_nc()
    in_maps = [
        {"x_line": xl[b], "d_line": dl[b], "wts": wts, "bias": bias_col}
        for b in range(B)
    ]
    res = run_bass_kernel_spmd(nc, in_maps, list(range(B)))

    out = np.empty((B, C, H, W), np.float32)
    for b in range(B):
        ol = res.results[b]["out_line"][:, :NP].astype(np.float32)
        out[b] = ol.reshape(C, Hp, Wp)[:, 1:257, 1:257]
    return out
